# revision 6
# baseline (speedup 1.0000x reference)
"""Trainium2 Bass kernel for CalculateSLayer GNN message passing.

Computes, for adj [L, L, 2] f32 and h [L, D] f32 with A = adj.sum(-1):
    h_in[j, d]  = sum_i A[i, j] * h[i, d]   (= A.T @ h)
    h_out[i, d] = sum_j A[i, j] * h[j, d]   (= A @ h)

Sharding: rows of A across 8 NeuronCores. Core m holds A[m*512:(m+1)*512, :]:
  - h_out rows are fully local:      h_out_blk = A_blk @ h
  - h_in is a partial sum per core:  p_in      = A_blk.T @ h_blk
    (the 8 partials are summed on the host during unshard)

On-chip per core: DMA the adj row block in j-windows, edge-sum to bf16 A
tiles on VectorE, then TensorE does both GEMMs (PSUM fp32 accumulation).
A-tiles are PE-transposed for the j-contraction (h_out) GEMM.
"""

import numpy as np

L = 4096
D = 150
NCORES = 8
R = L // NCORES  # 512 rows per core
P = 128  # partitions
IC = R // P  # 4 i-chunks per core
JW = 512  # j-window width
NW = L // JW  # 8 windows
JCW = JW // P  # 4 j-chunks per window
NJC = L // P  # 32 j-chunks total

_NC_CACHE = {}
LAST_RESULTS = None


def _ensure_ntff_hook():
    """Register the axon NTFF profile hook if the image's antenv lacks it.

    The boot shim (trn_agent_boot.trn_boot) only registers the hook when
    ``antenv.axon_hooks`` is importable; on images where it isn't, tracing
    raises ModuleNotFoundError. Inject an equivalent in-memory module and
    register the ctypes-based hook against libaxon_pjrt.so.
    """
    import sys
    import types

    try:
        from antenv.axon_hooks import get_axon_ntff_profile_hook  # noqa: F401

        return
    except ImportError:
        pass

    mod = types.ModuleType("antenv.axon_hooks")
    _state = {"hook": None}
    mod.set_axon_ntff_profile_hook = lambda h: _state.__setitem__("hook", h)
    mod.get_axon_ntff_profile_hook = lambda: _state["hook"]
    sys.modules["antenv.axon_hooks"] = mod
    import antenv

    antenv.axon_hooks = mod

    so_path = "/opt/axon/libaxon_pjrt.so"
    try:
        from trn_agent_boot.trn_boot import _ntff_profile_via_ctypes

        hook = _ntff_profile_via_ctypes(so_path)
        if hook is not None:
            mod.set_axon_ntff_profile_hook(hook)
    except Exception:
        pass

    # artifact upload has no bucket in this container; make it a no-op
    try:
        from concourse import bass_utils

        bass_utils.upload_artifacts = lambda tmpdir: tmpdir
    except Exception:
        pass


def _build_nc():
    import concourse.bacc as bacc
    import concourse.tile as tile
    import concourse.mybir as mybir

    f32 = mybir.dt.float32
    bf16 = mybir.dt.bfloat16

    nc = bacc.Bacc(
        "TRN2", target_bir_lowering=False, debug=False, num_devices=NCORES
    )
    adj_d = nc.dram_tensor("adj_blk", [R, L, 2], f32, kind="ExternalInput").ap()
    h_d = nc.dram_tensor("h", [L, D], f32, kind="ExternalInput").ap()
    hb_d = nc.dram_tensor("h_blk", [R, D], f32, kind="ExternalInput").ap()
    # outputs are produced transposed: [D, ...]; the host transposes back
    pin_d = nc.dram_tensor("p_inT", [D, L], f32, kind="ExternalOutput").ap()
    hout_d = nc.dram_tensor("h_outT_blk", [D, R], f32, kind="ExternalOutput").ap()

    DT = ((0, 128), (128, D))  # d-tile splits (M <= 128)

    with tile.TileContext(nc) as tc:
        with (
            tc.tile_pool(name="const", bufs=1) as const_pool,
            tc.tile_pool(name="adj", bufs=3) as adj_pool,
            tc.tile_pool(name="abf", bufs=2) as abf_pool,
            tc.tile_pool(name="at", bufs=2) as at_pool,
            tc.tile_pool(name="pouts", bufs=2) as pout_pool,
            tc.tile_pool(name="pinps", bufs=2, space="PSUM") as pin_psum,
            tc.tile_pool(name="houtps", bufs=1, space="PSUM") as hout_psum,
        ):
            # full h, laid out [p, chunk, d] with j = chunk*128 + p
            h_sb = const_pool.tile([P, NJC, D], f32)
            nc.sync.dma_start(h_sb[:], h_d.rearrange("(c p) d -> p c d", p=P))
            h_bf = const_pool.tile([P, NJC, D], bf16)
            nc.vector.tensor_copy(h_bf[:], h_sb[:])

            # this core's row block of h, [p, ic, d] with i = ic*128 + p
            hb_sb = const_pool.tile([P, IC, D], f32)
            nc.sync.dma_start(hb_sb[:], hb_d.rearrange("(c p) d -> p c d", p=P))
            hb_bf = const_pool.tile([P, IC, D], bf16)
            nc.vector.tensor_copy(hb_bf[:], hb_sb[:])

            hout_ps = [
                hout_psum.tile([DT[t][1] - DT[t][0], R], f32, tag=f"ho{t}",
                               name=f"hout_ps{t}")
                for t in range(2)
            ]

            for w in range(NW):
                a_bf = []
                for ic in range(IC):
                    adj_t = adj_pool.tile([P, JW, 2], f32, tag=f"adj{ic}")
                    nc.sync.dma_start(
                        adj_t[:],
                        adj_d[ic * P : (ic + 1) * P, w * JW : (w + 1) * JW, :],
                    )
                    ab = abf_pool.tile([P, JW], bf16, tag=f"abf{ic}")
                    nc.vector.tensor_add(ab[:], adj_t[:, :, 0], adj_t[:, :, 1])
                    a_bf.append(ab)

                # xbar-transpose all 16 [128,128] A tiles of this window
                # (ACT's HWDGE queue; PE and the sync queue stay free)
                at_tiles = []
                for jc in range(JCW):
                    at_sb = at_pool.tile([P, R], bf16, tag=f"at{jc}")
                    for ic in range(IC):
                        nc.scalar.dma_start(
                            at_sb[:, ic * P : (ic + 1) * P],
                            a_bf[ic][:, jc * P : (jc + 1) * P],
                            transpose=True,
                        )
                    at_tiles.append(at_sb)

                # p_inT[d, j] += h_blk[i, d] * A_blk[i, j], contract i
                for t, (d0, dn) in enumerate(DT):
                    pt = pin_psum.tile([dn - d0, JW], f32, tag=f"pt{t}",
                                       name=f"pt{t}")
                    for ic in range(IC):
                        nc.tensor.matmul(
                            pt[:],
                            hb_bf[:, ic, d0:dn],
                            a_bf[ic][:],
                            start=(ic == 0),
                            stop=(ic == IC - 1),
                        )
                    po = pout_pool.tile([dn - d0, JW], f32, tag=f"po{t}",
                                        name=f"po{t}")
                    nc.vector.tensor_copy(po[:], pt[:])
                    nc.sync.dma_start(
                        pin_d[d0:dn, w * JW : (w + 1) * JW], po[:]
                    )

                # h_outT[d, i] += h[j, d] * A_blk[i, j], contract j
                for jc in range(JCW):
                    g = w * JCW + jc
                    for t, (d0, dn) in enumerate(DT):
                        nc.tensor.matmul(
                            hout_ps[t][:],
                            h_bf[:, g, d0:dn],
                            at_tiles[jc][:],
                            start=(g == 0),
                            stop=(g == NJC - 1),
                        )

            for t, (d0, dn) in enumerate(DT):
                ho = pout_pool.tile([dn - d0, R], f32, tag=f"hoev{t}",
                                    name=f"hoev{t}")
                nc.vector.tensor_copy(ho[:], hout_ps[t][:])
                nc.sync.dma_start(hout_d[d0:dn, :], ho[:])

    nc.compile()
    return nc


def _get_nc():
    if "nc" not in _NC_CACHE:
        _NC_CACHE["nc"] = _build_nc()
    return _NC_CACHE["nc"]


def _run_cores(adj, h, trace=False):
    from concourse.bass_utils import run_bass_kernel_spmd

    global LAST_RESULTS
    if trace:
        _ensure_ntff_hook()
    nc = _get_nc()
    in_maps = []
    for m in range(NCORES):
        in_maps.append(
            {
                "adj_blk": np.ascontiguousarray(adj[m * R : (m + 1) * R]),
                "h": h,
                "h_blk": np.ascontiguousarray(h[m * R : (m + 1) * R]),
            }
        )
    res = run_bass_kernel_spmd(
        nc, in_maps, core_ids=list(range(NCORES)), trace=trace
    )
    LAST_RESULTS = res
    return res


def kernel(unpreprocessed_unweight_adj_matrix, h):
    adj = np.ascontiguousarray(
        np.asarray(unpreprocessed_unweight_adj_matrix, dtype=np.float32)
    )
    h = np.ascontiguousarray(np.asarray(h, dtype=np.float32))
    res = _run_cores(adj, h)
    parts = res.results
    h_inT = np.zeros((D, L), dtype=np.float64)
    for r in parts:
        h_inT += r["p_inT"].astype(np.float64)
    h_out = np.concatenate(
        [np.asarray(r["h_outT_blk"]).T for r in parts], axis=0
    )
    return (
        np.ascontiguousarray(h_inT.T).astype(np.float32),
        np.ascontiguousarray(h_out, dtype=np.float32),
    )


# revision 10
# speedup vs baseline: 4.0135x; 4.0135x over previous
"""Trainium2 Bass kernel for CalculateSLayer GNN message passing.

Computes, for adj [L, L, 2] f32 and h [L, D] f32 with A = adj.sum(-1):
    h_in[j, d]  = sum_i A[i, j] * h[i, d]   (= A.T @ h)
    h_out[i, d] = sum_j A[i, j] * h[j, d]   (= A @ h)

Sharding: rows of A across 8 NeuronCores. Core m holds A[m*512:(m+1)*512, :]:
  - h_out rows are fully local:      h_out_blk = A_blk @ h
  - h_in is a partial sum per core:  p_in      = A_blk.T @ h_blk
    (the 8 partials are summed on the host during unshard)

On-chip per core: DMA the adj row block in j-windows, edge-sum to bf16 A
tiles on VectorE, then TensorE does both GEMMs (PSUM fp32 accumulation).
A-tiles are PE-transposed for the j-contraction (h_out) GEMM.
"""

import numpy as np

L = 4096
D = 150
NCORES = 8
R = L // NCORES  # 512 rows per core
P = 128  # partitions
IC = R // P  # 4 i-chunks per core
JW = 512  # j-window width
NW = L // JW  # 8 windows
JCW = JW // P  # 4 j-chunks per window
NJC = L // P  # 32 j-chunks total

_NC_CACHE = {}
LAST_RESULTS = None


def _ensure_ntff_hook():
    """Register the axon NTFF profile hook if the image's antenv lacks it.

    The boot shim (trn_agent_boot.trn_boot) only registers the hook when
    ``antenv.axon_hooks`` is importable; on images where it isn't, tracing
    raises ModuleNotFoundError. Inject an equivalent in-memory module and
    register the ctypes-based hook against libaxon_pjrt.so.
    """
    import sys
    import types

    try:
        from antenv.axon_hooks import get_axon_ntff_profile_hook  # noqa: F401

        return
    except ImportError:
        pass

    mod = types.ModuleType("antenv.axon_hooks")
    _state = {"hook": None}
    mod.set_axon_ntff_profile_hook = lambda h: _state.__setitem__("hook", h)
    mod.get_axon_ntff_profile_hook = lambda: _state["hook"]
    sys.modules["antenv.axon_hooks"] = mod
    import antenv

    antenv.axon_hooks = mod

    so_path = "/opt/axon/libaxon_pjrt.so"
    try:
        from trn_agent_boot.trn_boot import _ntff_profile_via_ctypes

        hook = _ntff_profile_via_ctypes(so_path)
        if hook is not None:
            mod.set_axon_ntff_profile_hook(hook)
    except Exception:
        pass

    # artifact upload has no bucket in this container; make it a no-op
    try:
        from concourse import bass_utils

        bass_utils.upload_artifacts = lambda tmpdir: tmpdir
    except Exception:
        pass


def _build_nc():
    import concourse.bacc as bacc
    import concourse.tile as tile
    import concourse.mybir as mybir
    from concourse.masks import make_identity

    f32 = mybir.dt.float32
    bf16 = mybir.dt.bfloat16

    nc = bacc.Bacc(
        "TRN2", target_bir_lowering=False, debug=False, num_devices=NCORES
    )
    adj_d = nc.dram_tensor("adj_blk", [R, L, 2], f32, kind="ExternalInput").ap()
    h_d = nc.dram_tensor("h", [L, D], f32, kind="ExternalInput").ap()
    hb_d = nc.dram_tensor("h_blk", [R, D], f32, kind="ExternalInput").ap()
    # outputs are produced transposed: [D, ...]; the host transposes back
    pin_d = nc.dram_tensor("p_inT", [D, L], f32, kind="ExternalOutput").ap()
    hout_d = nc.dram_tensor("h_outT_blk", [D, R], f32, kind="ExternalOutput").ap()

    DT = ((0, 128), (128, D))  # d-tile splits (M <= 128)

    with tile.TileContext(nc) as tc:
        with (
            tc.tile_pool(name="const", bufs=1) as const_pool,
            tc.tile_pool(name="adj", bufs=3) as adj_pool,
            tc.tile_pool(name="abf", bufs=2) as abf_pool,
            tc.tile_pool(name="at", bufs=2) as at_pool,
            tc.tile_pool(name="pouts", bufs=2) as pout_pool,
            tc.tile_pool(name="pinps", bufs=2, space="PSUM") as pin_psum,
            tc.tile_pool(name="atps", bufs=2, space="PSUM") as at_psum,
            tc.tile_pool(name="houtps", bufs=1, space="PSUM") as hout_psum,
        ):
            ident = const_pool.tile([P, P], bf16)
            make_identity(nc, ident[:])

            # full h, laid out [p, chunk, d] with j = chunk*128 + p
            h_sb = const_pool.tile([P, NJC, D], f32)
            nc.sync.dma_start(h_sb[:], h_d.rearrange("(c p) d -> p c d", p=P))
            h_bf = const_pool.tile([P, NJC, D], bf16)
            nc.vector.tensor_copy(h_bf[:], h_sb[:])

            # this core's row block of h, [p, ic, d] with i = ic*128 + p
            hb_sb = const_pool.tile([P, IC, D], f32)
            nc.sync.dma_start(hb_sb[:], hb_d.rearrange("(c p) d -> p c d", p=P))
            hb_bf = const_pool.tile([P, IC, D], bf16)
            nc.vector.tensor_copy(hb_bf[:], hb_sb[:])

            hout_ps = [
                hout_psum.tile([DT[t][1] - DT[t][0], R], f32, tag=f"ho{t}",
                               name=f"hout_ps{t}")
                for t in range(2)
            ]

            for w in range(NW):
                a_bf = []
                for ic in range(IC):
                    adj_t = adj_pool.tile([P, JW, 2], f32, tag=f"adj{ic}")
                    nc.sync.dma_start(
                        adj_t[:],
                        adj_d[ic * P : (ic + 1) * P, w * JW : (w + 1) * JW, :],
                    )
                    ab = abf_pool.tile([P, JW], bf16, tag=f"abf{ic}")
                    nc.vector.tensor_add(ab[:], adj_t[:, :, 0], adj_t[:, :, 1])
                    a_bf.append(ab)

                # PE-transpose all 16 [128,128] A tiles of this window,
                # 4 per PSUM tile; ScalarE evicts to SBUF
                at_tiles = []
                for jc in range(JCW):
                    at_ps = at_psum.tile([P, R], bf16, tag="atps",
                                         name=f"at_ps{jc}")
                    for ic in range(IC):
                        nc.tensor.transpose(
                            at_ps[:, ic * P : (ic + 1) * P],
                            a_bf[ic][:, jc * P : (jc + 1) * P],
                            ident[:],
                        )
                    at_sb = at_pool.tile([P, R], bf16, tag=f"at{jc}")
                    nc.scalar.copy(at_sb[:], at_ps[:])
                    at_tiles.append(at_sb)

                # p_inT[d, j] += h_blk[i, d] * A_blk[i, j], contract i
                for t, (d0, dn) in enumerate(DT):
                    pt = pin_psum.tile([dn - d0, JW], f32, tag=f"pt{t}",
                                       name=f"pt{t}")
                    for ic in range(IC):
                        nc.tensor.matmul(
                            pt[:],
                            hb_bf[:, ic, d0:dn],
                            a_bf[ic][:],
                            start=(ic == 0),
                            stop=(ic == IC - 1),
                        )
                    po = pout_pool.tile([dn - d0, JW], f32, tag=f"po{t}",
                                        name=f"po{t}")
                    nc.vector.tensor_copy(po[:], pt[:])
                    nc.sync.dma_start(
                        pin_d[d0:dn, w * JW : (w + 1) * JW], po[:]
                    )

                # h_outT[d, i] += h[j, d] * A_blk[i, j], contract j
                for jc in range(JCW):
                    g = w * JCW + jc
                    for t, (d0, dn) in enumerate(DT):
                        nc.tensor.matmul(
                            hout_ps[t][:],
                            h_bf[:, g, d0:dn],
                            at_tiles[jc][:],
                            start=(g == 0),
                            stop=(g == NJC - 1),
                        )

            for t, (d0, dn) in enumerate(DT):
                ho = pout_pool.tile([dn - d0, R], f32, tag=f"hoev{t}",
                                    name=f"hoev{t}")
                nc.vector.tensor_copy(ho[:], hout_ps[t][:])
                nc.sync.dma_start(hout_d[d0:dn, :], ho[:])

    nc.compile()
    return nc


def _get_nc():
    if "nc" not in _NC_CACHE:
        _NC_CACHE["nc"] = _build_nc()
    return _NC_CACHE["nc"]


def _run_cores(adj, h, trace=False):
    from concourse.bass_utils import run_bass_kernel_spmd

    global LAST_RESULTS
    if trace:
        _ensure_ntff_hook()
    nc = _get_nc()
    in_maps = []
    for m in range(NCORES):
        in_maps.append(
            {
                "adj_blk": np.ascontiguousarray(adj[m * R : (m + 1) * R]),
                "h": h,
                "h_blk": np.ascontiguousarray(h[m * R : (m + 1) * R]),
            }
        )
    res = run_bass_kernel_spmd(
        nc, in_maps, core_ids=list(range(NCORES)), trace=trace
    )
    LAST_RESULTS = res
    return res


def kernel(unpreprocessed_unweight_adj_matrix, h):
    adj = np.ascontiguousarray(
        np.asarray(unpreprocessed_unweight_adj_matrix, dtype=np.float32)
    )
    h = np.ascontiguousarray(np.asarray(h, dtype=np.float32))
    res = _run_cores(adj, h)
    parts = res.results
    h_inT = np.zeros((D, L), dtype=np.float64)
    for r in parts:
        h_inT += r["p_inT"].astype(np.float64)
    h_out = np.concatenate(
        [np.asarray(r["h_outT_blk"]).T for r in parts], axis=0
    )
    return (
        np.ascontiguousarray(h_inT.T).astype(np.float32),
        np.ascontiguousarray(h_out, dtype=np.float32),
    )


# revision 11
# speedup vs baseline: 4.2400x; 1.0564x over previous
"""Trainium2 Bass kernel for CalculateSLayer GNN message passing.

Computes, for adj [L, L, 2] f32 and h [L, D] f32 with A = adj.sum(-1):
    h_in[j, d]  = sum_i A[i, j] * h[i, d]   (= A.T @ h)
    h_out[i, d] = sum_j A[i, j] * h[j, d]   (= A @ h)

Sharding: rows of A across 8 NeuronCores. Core m holds A[m*512:(m+1)*512, :]:
  - h_out rows are fully local:      h_out_blk = A_blk @ h
  - h_in is a partial sum per core:  p_in      = A_blk.T @ h_blk
    (the 8 partials are summed on the host during unshard)

On-chip per core: DMA the adj row block in j-windows, edge-sum to bf16 A
tiles on VectorE, then TensorE does both GEMMs (PSUM fp32 accumulation).
A-tiles are PE-transposed for the j-contraction (h_out) GEMM.
"""

import numpy as np

L = 4096
D = 150
NCORES = 8
R = L // NCORES  # 512 rows per core
P = 128  # partitions
IC = R // P  # 4 i-chunks per core
JW = 512  # j-window width
NW = L // JW  # 8 windows
JCW = JW // P  # 4 j-chunks per window
NJC = L // P  # 32 j-chunks total

_NC_CACHE = {}
LAST_RESULTS = None


def _ensure_ntff_hook():
    """Register the axon NTFF profile hook if the image's antenv lacks it.

    The boot shim (trn_agent_boot.trn_boot) only registers the hook when
    ``antenv.axon_hooks`` is importable; on images where it isn't, tracing
    raises ModuleNotFoundError. Inject an equivalent in-memory module and
    register the ctypes-based hook against libaxon_pjrt.so.
    """
    import sys
    import types

    try:
        from antenv.axon_hooks import get_axon_ntff_profile_hook  # noqa: F401

        return
    except ImportError:
        pass

    mod = types.ModuleType("antenv.axon_hooks")
    _state = {"hook": None}
    mod.set_axon_ntff_profile_hook = lambda h: _state.__setitem__("hook", h)
    mod.get_axon_ntff_profile_hook = lambda: _state["hook"]
    sys.modules["antenv.axon_hooks"] = mod
    import antenv

    antenv.axon_hooks = mod

    so_path = "/opt/axon/libaxon_pjrt.so"
    try:
        from trn_agent_boot.trn_boot import _ntff_profile_via_ctypes

        hook = _ntff_profile_via_ctypes(so_path)
        if hook is not None:
            mod.set_axon_ntff_profile_hook(hook)
    except Exception:
        pass

    # artifact upload has no bucket in this container; make it a no-op
    try:
        from concourse import bass_utils

        bass_utils.upload_artifacts = lambda tmpdir: tmpdir
    except Exception:
        pass


def _build_nc():
    import concourse.bacc as bacc
    import concourse.tile as tile
    import concourse.mybir as mybir
    from concourse.masks import make_identity

    f32 = mybir.dt.float32
    bf16 = mybir.dt.bfloat16

    nc = bacc.Bacc(
        "TRN2", target_bir_lowering=False, debug=False, num_devices=NCORES
    )
    adj_d = nc.dram_tensor("adj_blk", [R, L, 2], f32, kind="ExternalInput").ap()
    h_d = nc.dram_tensor("h", [L, D], f32, kind="ExternalInput").ap()
    hb_d = nc.dram_tensor("h_blk", [R, D], f32, kind="ExternalInput").ap()
    # outputs are produced transposed: [D, ...]; the host transposes back
    pin_d = nc.dram_tensor("p_inT", [D, L], f32, kind="ExternalOutput").ap()
    hout_d = nc.dram_tensor("h_outT_blk", [D, R], f32, kind="ExternalOutput").ap()

    DT = ((0, 128), (128, D))  # d-tile splits (M <= 128)

    with tile.TileContext(nc) as tc:
        with (
            tc.tile_pool(name="const", bufs=1) as const_pool,
            tc.tile_pool(name="adj", bufs=4) as adj_pool,
            tc.tile_pool(name="abf", bufs=2) as abf_pool,
            tc.tile_pool(name="at", bufs=2) as at_pool,
            tc.tile_pool(name="pouts", bufs=2) as pout_pool,
            tc.tile_pool(name="pinps", bufs=2, space="PSUM") as pin_psum,
            tc.tile_pool(name="atps", bufs=2, space="PSUM") as at_psum,
            tc.tile_pool(name="houtps", bufs=1, space="PSUM") as hout_psum,
        ):
            ident = const_pool.tile([P, P], bf16)
            make_identity(nc, ident[:])

            # full h, laid out [p, chunk, d] with j = chunk*128 + p
            h_sb = const_pool.tile([P, NJC, D], f32)
            nc.scalar.dma_start(h_sb[:], h_d.rearrange("(c p) d -> p c d", p=P))
            h_bf = const_pool.tile([P, NJC, D], bf16)
            nc.vector.tensor_copy(h_bf[:], h_sb[:])

            # this core's row block of h, [p, ic, d] with i = ic*128 + p
            hb_sb = const_pool.tile([P, IC, D], f32)
            nc.scalar.dma_start(hb_sb[:], hb_d.rearrange("(c p) d -> p c d", p=P))
            hb_bf = const_pool.tile([P, IC, D], bf16)
            nc.vector.tensor_copy(hb_bf[:], hb_sb[:])

            hout_ps = [
                hout_psum.tile([DT[t][1] - DT[t][0], R], f32, tag=f"ho{t}",
                               name=f"hout_ps{t}")
                for t in range(2)
            ]

            for w in range(NW):
                a_bf = []
                for ic in range(IC):
                    adj_t = adj_pool.tile([P, JW, 2], f32, tag=f"adj{ic}")
                    nc.sync.dma_start(
                        adj_t[:],
                        adj_d[ic * P : (ic + 1) * P, w * JW : (w + 1) * JW, :],
                    )
                    ab = abf_pool.tile([P, JW], bf16, tag=f"abf{ic}")
                    nc.vector.tensor_add(ab[:], adj_t[:, :, 0], adj_t[:, :, 1])
                    a_bf.append(ab)

                # PE-transpose all 16 [128,128] A tiles of this window,
                # 4 per PSUM tile; ScalarE evicts to SBUF
                at_tiles = []
                for jc in range(JCW):
                    at_ps = at_psum.tile([P, R], bf16, tag="atps",
                                         name=f"at_ps{jc}")
                    for ic in range(IC):
                        nc.tensor.transpose(
                            at_ps[:, ic * P : (ic + 1) * P],
                            a_bf[ic][:, jc * P : (jc + 1) * P],
                            ident[:],
                        )
                    at_sb = at_pool.tile([P, R], bf16, tag=f"at{jc}")
                    nc.scalar.copy(at_sb[:], at_ps[:])
                    at_tiles.append(at_sb)

                # p_inT[d, j] += h_blk[i, d] * A_blk[i, j], contract i
                for t, (d0, dn) in enumerate(DT):
                    pt = pin_psum.tile([dn - d0, JW], f32, tag=f"pt{t}",
                                       name=f"pt{t}")
                    for ic in range(IC):
                        nc.tensor.matmul(
                            pt[:],
                            hb_bf[:, ic, d0:dn],
                            a_bf[ic][:],
                            start=(ic == 0),
                            stop=(ic == IC - 1),
                        )
                    po = pout_pool.tile([dn - d0, JW], f32, tag=f"po{t}",
                                        name=f"po{t}")
                    nc.vector.tensor_copy(po[:], pt[:])
                    nc.scalar.dma_start(
                        pin_d[d0:dn, w * JW : (w + 1) * JW], po[:]
                    )

                # h_outT[d, i] += h[j, d] * A_blk[i, j], contract j
                for jc in range(JCW):
                    g = w * JCW + jc
                    for t, (d0, dn) in enumerate(DT):
                        nc.tensor.matmul(
                            hout_ps[t][:],
                            h_bf[:, g, d0:dn],
                            at_tiles[jc][:],
                            start=(g == 0),
                            stop=(g == NJC - 1),
                        )

            for t, (d0, dn) in enumerate(DT):
                ho = pout_pool.tile([dn - d0, R], f32, tag=f"hoev{t}",
                                    name=f"hoev{t}")
                nc.vector.tensor_copy(ho[:], hout_ps[t][:])
                nc.scalar.dma_start(hout_d[d0:dn, :], ho[:])

    nc.compile()
    return nc


def _get_nc():
    if "nc" not in _NC_CACHE:
        _NC_CACHE["nc"] = _build_nc()
    return _NC_CACHE["nc"]


def _run_cores(adj, h, trace=False):
    from concourse.bass_utils import run_bass_kernel_spmd

    global LAST_RESULTS
    if trace:
        _ensure_ntff_hook()
    nc = _get_nc()
    in_maps = []
    for m in range(NCORES):
        in_maps.append(
            {
                "adj_blk": np.ascontiguousarray(adj[m * R : (m + 1) * R]),
                "h": h,
                "h_blk": np.ascontiguousarray(h[m * R : (m + 1) * R]),
            }
        )
    res = run_bass_kernel_spmd(
        nc, in_maps, core_ids=list(range(NCORES)), trace=trace
    )
    LAST_RESULTS = res
    return res


def kernel(unpreprocessed_unweight_adj_matrix, h):
    adj = np.ascontiguousarray(
        np.asarray(unpreprocessed_unweight_adj_matrix, dtype=np.float32)
    )
    h = np.ascontiguousarray(np.asarray(h, dtype=np.float32))
    res = _run_cores(adj, h)
    parts = res.results
    h_inT = np.zeros((D, L), dtype=np.float64)
    for r in parts:
        h_inT += r["p_inT"].astype(np.float64)
    h_out = np.concatenate(
        [np.asarray(r["h_outT_blk"]).T for r in parts], axis=0
    )
    return (
        np.ascontiguousarray(h_inT.T).astype(np.float32),
        np.ascontiguousarray(h_out, dtype=np.float32),
    )


# revision 12
# speedup vs baseline: 4.3744x; 1.0317x over previous
"""Trainium2 Bass kernel for CalculateSLayer GNN message passing.

Computes, for adj [L, L, 2] f32 and h [L, D] f32 with A = adj.sum(-1):
    h_in[j, d]  = sum_i A[i, j] * h[i, d]   (= A.T @ h)
    h_out[i, d] = sum_j A[i, j] * h[j, d]   (= A @ h)

Sharding: rows of A across 8 NeuronCores. Core m holds A[m*512:(m+1)*512, :]:
  - h_out rows are fully local:      h_out_blk = A_blk @ h
  - h_in is a partial sum per core:  p_in      = A_blk.T @ h_blk
    (the 8 partials are summed on the host during unshard)

On-chip per core: DMA the adj row block in j-windows, edge-sum to bf16 A
tiles on VectorE, then TensorE does both GEMMs (PSUM fp32 accumulation).
A-tiles are PE-transposed for the j-contraction (h_out) GEMM.
"""

import numpy as np

L = 4096
D = 150
NCORES = 8
R = L // NCORES  # 512 rows per core
P = 128  # partitions
IC = R // P  # 4 i-chunks per core
JW = 512  # j-window width
NW = L // JW  # 8 windows
JCW = JW // P  # 4 j-chunks per window
NJC = L // P  # 32 j-chunks total

_NC_CACHE = {}
LAST_RESULTS = None


def _ensure_ntff_hook():
    """Register the axon NTFF profile hook if the image's antenv lacks it.

    The boot shim (trn_agent_boot.trn_boot) only registers the hook when
    ``antenv.axon_hooks`` is importable; on images where it isn't, tracing
    raises ModuleNotFoundError. Inject an equivalent in-memory module and
    register the ctypes-based hook against libaxon_pjrt.so.
    """
    import sys
    import types

    try:
        from antenv.axon_hooks import get_axon_ntff_profile_hook  # noqa: F401

        return
    except ImportError:
        pass

    mod = types.ModuleType("antenv.axon_hooks")
    _state = {"hook": None}
    mod.set_axon_ntff_profile_hook = lambda h: _state.__setitem__("hook", h)
    mod.get_axon_ntff_profile_hook = lambda: _state["hook"]
    sys.modules["antenv.axon_hooks"] = mod
    import antenv

    antenv.axon_hooks = mod

    so_path = "/opt/axon/libaxon_pjrt.so"
    try:
        from trn_agent_boot.trn_boot import _ntff_profile_via_ctypes

        hook = _ntff_profile_via_ctypes(so_path)
        if hook is not None:
            mod.set_axon_ntff_profile_hook(hook)
    except Exception:
        pass

    # artifact upload has no bucket in this container; make it a no-op
    try:
        from concourse import bass_utils

        bass_utils.upload_artifacts = lambda tmpdir: tmpdir
    except Exception:
        pass


def _build_nc():
    import concourse.bacc as bacc
    import concourse.tile as tile
    import concourse.mybir as mybir
    from concourse.masks import make_identity

    f32 = mybir.dt.float32
    bf16 = mybir.dt.bfloat16

    nc = bacc.Bacc(
        "TRN2", target_bir_lowering=False, debug=False, num_devices=NCORES
    )
    adj_d = nc.dram_tensor("adj_blk", [R, L, 2], f32, kind="ExternalInput").ap()
    h_d = nc.dram_tensor("h", [L, D], f32, kind="ExternalInput").ap()
    hb_d = nc.dram_tensor("h_blk", [R, D], f32, kind="ExternalInput").ap()
    # outputs are produced transposed: [D, ...]; the host transposes back
    pin_d = nc.dram_tensor("p_inT", [D, L], bf16, kind="ExternalOutput").ap()
    hout_d = nc.dram_tensor("h_outT_blk", [D, R], bf16, kind="ExternalOutput").ap()

    DT = ((0, 128), (128, D))  # d-tile splits (M <= 128)

    with tile.TileContext(nc) as tc:
        with (
            tc.tile_pool(name="const", bufs=1) as const_pool,
            tc.tile_pool(name="adj", bufs=4) as adj_pool,
            tc.tile_pool(name="abf", bufs=3) as abf_pool,
            tc.tile_pool(name="at", bufs=3) as at_pool,
            tc.tile_pool(name="pouts", bufs=3) as pout_pool,
            tc.tile_pool(name="pinps", bufs=2, space="PSUM") as pin_psum,
            tc.tile_pool(name="atps", bufs=2, space="PSUM") as at_psum,
            tc.tile_pool(name="houtps", bufs=1, space="PSUM") as hout_psum,
        ):
            ident = const_pool.tile([P, P], bf16)
            make_identity(nc, ident[:])

            # full h, laid out [p, chunk, d] with j = chunk*128 + p
            h_sb = const_pool.tile([P, NJC, D], f32)
            nc.scalar.dma_start(h_sb[:], h_d.rearrange("(c p) d -> p c d", p=P))
            h_bf = const_pool.tile([P, NJC, D], bf16)
            nc.vector.tensor_copy(h_bf[:], h_sb[:])

            # this core's row block of h, [p, ic, d] with i = ic*128 + p
            hb_sb = const_pool.tile([P, IC, D], f32)
            nc.scalar.dma_start(hb_sb[:], hb_d.rearrange("(c p) d -> p c d", p=P))
            hb_bf = const_pool.tile([P, IC, D], bf16)
            nc.vector.tensor_copy(hb_bf[:], hb_sb[:])

            hout_ps = [
                hout_psum.tile([DT[t][1] - DT[t][0], R], f32, tag=f"ho{t}",
                               name=f"hout_ps{t}")
                for t in range(2)
            ]

            for w in range(NW):
                a_bf = []
                for ic in range(IC):
                    adj_t = adj_pool.tile([P, JW, 2], f32, tag=f"adj{ic}")
                    nc.sync.dma_start(
                        adj_t[:],
                        adj_d[ic * P : (ic + 1) * P, w * JW : (w + 1) * JW, :],
                    )
                    ab = abf_pool.tile([P, JW], bf16, tag=f"abf{ic}")
                    nc.vector.tensor_add(ab[:], adj_t[:, :, 0], adj_t[:, :, 1])
                    a_bf.append(ab)

                # PE-transpose all 16 [128,128] A tiles of this window,
                # 4 per PSUM tile; ScalarE evicts to SBUF
                at_tiles = []
                for jc in range(JCW):
                    at_ps = at_psum.tile([P, R], bf16, tag="atps",
                                         name=f"at_ps{jc}")
                    for ic in range(IC):
                        nc.tensor.transpose(
                            at_ps[:, ic * P : (ic + 1) * P],
                            a_bf[ic][:, jc * P : (jc + 1) * P],
                            ident[:],
                        )
                    at_sb = at_pool.tile([P, R], bf16, tag=f"at{jc}")
                    nc.scalar.copy(at_sb[:], at_ps[:])
                    at_tiles.append(at_sb)

                # p_inT[d, j] += h_blk[i, d] * A_blk[i, j], contract i
                for t, (d0, dn) in enumerate(DT):
                    pt = pin_psum.tile([dn - d0, JW], f32, tag=f"pt{t}",
                                       name=f"pt{t}")
                    for ic in range(IC):
                        nc.tensor.matmul(
                            pt[:],
                            hb_bf[:, ic, d0:dn],
                            a_bf[ic][:],
                            start=(ic == 0),
                            stop=(ic == IC - 1),
                        )
                    po = pout_pool.tile([dn - d0, JW], bf16, tag=f"po{t}",
                                        name=f"po{t}")
                    nc.vector.tensor_copy(po[:], pt[:])
                    nc.scalar.dma_start(
                        pin_d[d0:dn, w * JW : (w + 1) * JW], po[:]
                    )

                # h_outT[d, i] += h[j, d] * A_blk[i, j], contract j
                for jc in range(JCW):
                    g = w * JCW + jc
                    for t, (d0, dn) in enumerate(DT):
                        nc.tensor.matmul(
                            hout_ps[t][:],
                            h_bf[:, g, d0:dn],
                            at_tiles[jc][:],
                            start=(g == 0),
                            stop=(g == NJC - 1),
                        )

            for t, (d0, dn) in enumerate(DT):
                ho = pout_pool.tile([dn - d0, R], bf16, tag=f"hoev{t}",
                                    name=f"hoev{t}")
                nc.vector.tensor_copy(ho[:], hout_ps[t][:])
                nc.scalar.dma_start(hout_d[d0:dn, :], ho[:])

    nc.compile()
    return nc


def _get_nc():
    if "nc" not in _NC_CACHE:
        _NC_CACHE["nc"] = _build_nc()
    return _NC_CACHE["nc"]


def _run_cores(adj, h, trace=False):
    from concourse.bass_utils import run_bass_kernel_spmd

    global LAST_RESULTS
    if trace:
        _ensure_ntff_hook()
    nc = _get_nc()
    in_maps = []
    for m in range(NCORES):
        in_maps.append(
            {
                "adj_blk": np.ascontiguousarray(adj[m * R : (m + 1) * R]),
                "h": h,
                "h_blk": np.ascontiguousarray(h[m * R : (m + 1) * R]),
            }
        )
    res = run_bass_kernel_spmd(
        nc, in_maps, core_ids=list(range(NCORES)), trace=trace
    )
    LAST_RESULTS = res
    return res


def kernel(unpreprocessed_unweight_adj_matrix, h):
    adj = np.ascontiguousarray(
        np.asarray(unpreprocessed_unweight_adj_matrix, dtype=np.float32)
    )
    h = np.ascontiguousarray(np.asarray(h, dtype=np.float32))
    res = _run_cores(adj, h)
    parts = res.results
    h_inT = np.zeros((D, L), dtype=np.float64)
    for r in parts:
        h_inT += np.asarray(r["p_inT"], dtype=np.float32).astype(np.float64)
    h_out = np.concatenate(
        [np.asarray(r["h_outT_blk"], dtype=np.float32).T for r in parts], axis=0
    )
    return (
        np.ascontiguousarray(h_inT.T).astype(np.float32),
        np.ascontiguousarray(h_out, dtype=np.float32),
    )
